# revision 5
# baseline (speedup 1.0000x reference)
"""AttnBlock (GroupNorm + single-head attention over HW pixels + proj + residual)
on 8 trn2 NeuronCores.

Sharding: core i handles batch b = i//2, query-half h = i%2 (2048 of 4096 pixels).
Each core recomputes GroupNorm and full K/V for its image (no collectives).
The host rolls the pixel axis per core so queries are always columns [0, 2048):
attention is permutation-invariant over keys and GroupNorm over pixels.

Math notes:
  - k_b drops out of softmax (it only adds a per-query constant to scores).
  - v_b and proj_b fold into the residual on the host:
      x + proj_w @ (attn @ (v + v_b)) + proj_b = x + proj_w @ (attn@v) + (proj_b + proj_w@v_b)
    because softmax rows sum to 1.
  - Scores are O(1) for this distribution, so exp() without max-subtraction is safe.
  - Softmax layout: S^T is computed (keys on partitions) so exp goes PSUM->SBUF on
    ScalarE with no transposes; key-sums come from a ones-vector matmul on the PE;
    the 1/sum normalization folds into the PV PSUM->SBUF cast.
"""

from contextlib import ExitStack

import ml_dtypes
import numpy as np

import concourse.bacc as bacc
import concourse.tile as tile
from concourse import mybir
from concourse.bass_utils import run_bass_kernel_spmd

BF16 = mybir.dt.bfloat16
F32 = mybir.dt.float32
AX = mybir.AxisListType
OP = mybir.AluOpType
AF = mybir.ActivationFunctionType

C = 512
N = 4096
NQ = 2048  # queries per core
P = 128
CT = C // P  # 4 channel part-tiles
JT = N // P  # 32 key tiles
NCH = NQ // 512  # 4 query chunks of 512
GSIZE = 16  # channels per group
NGROUPS = 32
EPS = 1e-6
SCALE = float(C) ** -0.5

_cache = {}


def build_program():
    nc = bacc.Bacc("TRN2", target_bir_lowering=False, debug=False, num_devices=8)

    xb = nc.declare_dram_parameter("xb", [C, N], BF16, isOutput=False)
    xr = nc.declare_dram_parameter("xr", [C, NQ], F32, isOutput=False)
    wq = nc.declare_dram_parameter("wq", [C, C], BF16, isOutput=False)
    wk = nc.declare_dram_parameter("wk", [C, C], BF16, isOutput=False)
    wv = nc.declare_dram_parameter("wv", [C, C], BF16, isOutput=False)
    wp = nc.declare_dram_parameter("wp", [C, C], BF16, isOutput=False)
    qb = nc.declare_dram_parameter("qb", [C, 1], F32, isOutput=False)
    gw = nc.declare_dram_parameter("gw", [C, 1], F32, isOutput=False)
    gb = nc.declare_dram_parameter("gb", [C, 1], F32, isOutput=False)
    gs = nc.declare_dram_parameter("gs", [CT, P, NGROUPS], F32, isOutput=False)
    out = nc.declare_dram_parameter("out", [C, NQ], F32, isOutput=True)

    with tile.TileContext(nc) as tc, ExitStack() as ctx:
        # ---- persistent tiles -------------------------------------------------
        wpool = ctx.enter_context(tc.tile_pool(name="w", bufs=4 * CT))
        kpool = ctx.enter_context(tc.tile_pool(name="k", bufs=CT))
        qpool = ctx.enter_context(tc.tile_pool(name="q", bufs=CT))
        vpool = ctx.enter_context(tc.tile_pool(name="v", bufs=JT))
        cpool = ctx.enter_context(tc.tile_pool(name="c", bufs=1))
        spool = ctx.enter_context(tc.tile_pool(name="s", bufs=4 * CT))

        wts = {}
        for nm, src in (("wq", wq), ("wk", wk), ("wv", wv), ("wp", wp)):
            tl = []
            for ci in range(CT):
                t = wpool.tile([P, C], BF16, tag="w")
                nc.sync.dma_start(out=t[:], in_=src[ci * P : (ci + 1) * P, :])
                tl.append(t)
            wts[nm] = tl

        qbt, gwt, gbt = [], [], []
        for ci in range(CT):
            sl = slice(ci * P, (ci + 1) * P)
            t = spool.tile([P, 1], F32, tag="qbt")
            nc.sync.dma_start(out=t[:], in_=qb[sl, :])
            qbt.append(t)
            t = spool.tile([P, 1], F32, tag="gwt")
            nc.sync.dma_start(out=t[:], in_=gw[sl, :])
            gwt.append(t)
            t = spool.tile([P, 1], F32, tag="gbt")
            nc.sync.dma_start(out=t[:], in_=gb[sl, :])
            gbt.append(t)

        ones_bf = cpool.tile([P, 1], BF16, tag="ones")
        nc.vector.memset(ones_bf, 1.0)

        kt = [kpool.tile([P, N], BF16, tag="kt", name=f"kt{i}") for i in range(CT)]
        qt = [qpool.tile([P, NQ], BF16, tag="qt", name=f"qt{i}") for i in range(CT)]
        vt = [vpool.tile([P, C], BF16, tag="vt", name=f"vt{i}") for i in range(JT)]

        # ---- phase 1+2: GroupNorm -> h (bf16, in place), then Q/K/V ----------
        with tc.tile_pool(name="gn", bufs=CT) as gnpool, \
             tc.tile_pool(name="gnt", bufs=2) as gntpool, \
             tc.tile_pool(name="gns", bufs=16) as gnspool, \
             tc.tile_pool(name="gnp", bufs=2, space="PSUM") as gnpsum:
            xt = []
            gst = []
            xsum, xsq = [], []
            for ci in range(CT):
                t = gnpool.tile([P, N], BF16, tag="xt")
                nc.sync.dma_start(out=t[:], in_=xb[ci * P : (ci + 1) * P, :])
                xt.append(t)
                g = gnspool.tile([P, NGROUPS], F32, tag="gst")
                nc.sync.dma_start(out=g[:], in_=gs[ci, :, :])
                gst.append(g)

                xs = gnspool.tile([P, 1], F32, tag="xsum")
                nc.vector.reduce_sum(out=xs[:], in_=t[:], axis=AX.X)
                xsum.append(xs)

                sq = gntpool.tile([P, N], BF16, tag="sq")
                s2 = gnspool.tile([P, 1], F32, tag="xsq")
                nc.scalar.activation(out=sq[:], in_=t[:], func=AF.Square,
                                     accum_out=s2[:])
                xsq.append(s2)

            psums = gnpsum.tile([1, NGROUPS], F32, tag="psums")
            psq = gnpsum.tile([1, NGROUPS], F32, tag="psq")
            for ci in range(CT):
                nc.tensor.matmul(psums[:], lhsT=xsum[ci][:], rhs=gst[ci][:],
                                 start=(ci == 0), stop=(ci == CT - 1))
            for ci in range(CT):
                nc.tensor.matmul(psq[:], lhsT=xsq[ci][:], rhs=gst[ci][:],
                                 start=(ci == 0), stop=(ci == CT - 1))

            inv_n = 1.0 / (GSIZE * N)
            srow = gnspool.tile([1, 2 * NGROUPS], F32, tag="srow")
            mean = srow[:, 0:NGROUPS]
            rstd = srow[:, NGROUPS : 2 * NGROUPS]
            nc.vector.tensor_scalar_mul(out=mean, in0=psums[:], scalar1=inv_n)
            nc.vector.tensor_scalar_mul(out=rstd, in0=psq[:], scalar1=inv_n)
            msq = gnspool.tile([1, NGROUPS], F32, tag="msq")
            nc.vector.tensor_mul(out=msq[:], in0=mean, in1=mean)
            nc.vector.tensor_sub(out=rstd, in0=rstd, in1=msq[:])
            epst = gnspool.tile([1, 1], F32, tag="epst")
            nc.vector.memset(epst, EPS)
            nc.scalar.activation(out=rstd, in_=rstd, func=AF.Sqrt, bias=epst[:])
            nc.vector.reciprocal(out=rstd, in_=rstd)

            bc = gnspool.tile([P, 2 * NGROUPS], F32, tag="bc")
            nc.gpsimd.partition_broadcast(bc[:], srow[:], channels=P)

            junk = gnspool.tile([P, NGROUPS], F32, tag="junk")
            for ci in range(CT):
                mpc = gnspool.tile([P, 1], F32, tag="mpc")
                spc = gnspool.tile([P, 1], F32, tag="spc")
                nc.vector.tensor_mul(out=junk[:], in0=bc[:, 0:NGROUPS],
                                     in1=gst[ci][:])
                nc.vector.reduce_sum(out=mpc[:], in_=junk[:], axis=AX.X)
                jnk2 = gnspool.tile([P, NGROUPS], F32, tag="jnk2")
                nc.vector.tensor_mul(out=jnk2[:], in0=bc[:, NGROUPS : 2 * NGROUPS],
                                     in1=gst[ci][:])
                nc.vector.reduce_sum(out=spc[:], in_=jnk2[:], axis=AX.X)
                # s = rstd*gamma ; t = beta - mean*s ; h = x*s + t
                sc = gnspool.tile([P, 1], F32, tag="sc")
                nc.vector.tensor_mul(out=sc[:], in0=spc[:], in1=gwt[ci][:])
                u = gnspool.tile([P, 1], F32, tag="u")
                nc.vector.tensor_mul(out=u[:], in0=mpc[:], in1=sc[:])
                tb = gnspool.tile([P, 1], F32, tag="tb")
                nc.vector.tensor_sub(out=tb[:], in0=gbt[ci][:], in1=u[:])
                nc.vector.tensor_scalar(
                    out=xt[ci][:], in0=xt[ci][:], scalar1=sc[:], scalar2=tb[:],
                    op0=OP.mult, op1=OP.add,
                )

            # ---- Q/K/V projections (h lives in xt) ---------------------------
            with tc.tile_pool(name="pqkv", bufs=4, space="PSUM") as pqkv:
                for oi in range(CT):
                    osl = slice(oi * P, (oi + 1) * P)
                    for ni in range(N // 512):
                        nsl = slice(ni * 512, (ni + 1) * 512)
                        ps = pqkv.tile([P, 512], F32, tag="ps")
                        for ci in range(CT):
                            nc.tensor.matmul(ps[:], lhsT=wts["wk"][ci][:, osl],
                                             rhs=xt[ci][:, nsl],
                                             start=(ci == 0), stop=(ci == CT - 1))
                        nc.vector.tensor_copy(out=kt[oi][:, nsl], in_=ps[:])
                for oi in range(CT):
                    osl = slice(oi * P, (oi + 1) * P)
                    for ni in range(NCH):
                        nsl = slice(ni * 512, (ni + 1) * 512)
                        ps = pqkv.tile([P, 512], F32, tag="ps")
                        for ci in range(CT):
                            nc.tensor.matmul(ps[:], lhsT=wts["wq"][ci][:, osl],
                                             rhs=xt[ci][:, nsl],
                                             start=(ci == 0), stop=(ci == CT - 1))
                        nc.vector.tensor_scalar_add(out=qt[oi][:, nsl], in0=ps[:],
                                                    scalar1=qbt[oi][:])
                for ji in range(JT):
                    jsl = slice(ji * P, (ji + 1) * P)
                    ps = pqkv.tile([P, 512], F32, tag="ps")
                    for ci in range(CT):
                        nc.tensor.matmul(ps[:], lhsT=xt[ci][:, jsl],
                                         rhs=wts["wv"][ci][:],
                                         start=(ci == 0), stop=(ci == CT - 1))
                    nc.vector.tensor_copy(out=vt[ji][:], in_=ps[:])

        # ---- phase 3: attention + proj + residual ----------------------------
        with tc.tile_pool(name="xr", bufs=CT) as xrpool, \
             tc.tile_pool(name="et", bufs=JT) as epool, \
             tc.tile_pool(name="at", bufs=2 * CT) as apool, \
             tc.tile_pool(name="ot", bufs=4) as opool, \
             tc.tile_pool(name="rc", bufs=2) as rcpool, \
             tc.tile_pool(name="pss", bufs=2, space="PSUM") as pss_pool, \
             tc.tile_pool(name="pcs", bufs=2, space="PSUM") as pcs_pool, \
             tc.tile_pool(name="pso", bufs=2, space="PSUM") as pso_pool, \
             tc.tile_pool(name="psp", bufs=2, space="PSUM") as psp_pool:

            xrt = []
            for ci in range(CT):
                t = xrpool.tile([P, NQ], F32, tag="xrt")
                nc.sync.dma_start(out=t[:], in_=xr[ci * P : (ci + 1) * P, :])
                xrt.append(t)

            for ch in range(NCH):
                isl = slice(ch * 512, (ch + 1) * 512)

                et = []
                for ji in range(JT):
                    jsl = slice(ji * P, (ji + 1) * P)
                    ps = pss_pool.tile([P, 512], F32, tag="pss")
                    for ci in range(CT):
                        nc.tensor.matmul(ps[:], lhsT=kt[ci][:, jsl],
                                         rhs=qt[ci][:, isl],
                                         start=(ci == 0), stop=(ci == CT - 1))
                    e = epool.tile([P, 512], BF16, tag="et")
                    nc.scalar.activation(out=e[:], in_=ps[:], func=AF.Exp,
                                         scale=SCALE)
                    et.append(e)

                pcs = pcs_pool.tile([1, 512], F32, tag="pcs")
                for ji in range(JT):
                    nc.tensor.matmul(pcs[:], lhsT=ones_bf[:], rhs=et[ji][:],
                                     start=(ji == 0), stop=(ji == JT - 1))
                rc = rcpool.tile([1, 512], F32, tag="rc")
                nc.vector.reciprocal(out=rc[:], in_=pcs[:])
                rcb = rcpool.tile([P, 512], F32, tag="rcb")
                nc.gpsimd.partition_broadcast(rcb[:], rc[:], channels=P)

                at = []
                for oi in range(CT):
                    osl = slice(oi * P, (oi + 1) * P)
                    ps = pso_pool.tile([P, 512], F32, tag="pso")
                    for ji in range(JT):
                        nc.tensor.matmul(ps[:], lhsT=vt[ji][:, osl], rhs=et[ji][:],
                                         start=(ji == 0), stop=(ji == JT - 1))
                    a = apool.tile([P, 512], BF16, tag="at")
                    nc.vector.tensor_mul(out=a[:], in0=ps[:], in1=rcb[:])
                    at.append(a)

                for oi in range(CT):
                    osl = slice(oi * P, (oi + 1) * P)
                    ps = psp_pool.tile([P, 512], F32, tag="psp")
                    for ci in range(CT):
                        nc.tensor.matmul(ps[:], lhsT=wts["wp"][ci][:, osl],
                                         rhs=at[ci][:],
                                         start=(ci == 0), stop=(ci == CT - 1))
                    o = opool.tile([P, 512], F32, tag="ot")
                    nc.vector.tensor_add(out=o[:], in0=ps[:], in1=xrt[oi][:, isl])
                    nc.sync.dma_start(out=out[oi * P : (oi + 1) * P, isl], in_=o[:])

    nc.compile()
    return nc


def _prep_inputs(x, gn_g, gn_b, q_w, q_b, k_w, k_b, v_w, v_b, proj_w, proj_b):
    bf = ml_dtypes.bfloat16
    B = x.shape[0]
    xf = np.ascontiguousarray(x.reshape(B, C, N), dtype=np.float32)
    pbe = (proj_b + proj_w.astype(np.float64) @ v_b.astype(np.float64)).astype(
        np.float32
    )

    wqT = np.ascontiguousarray(q_w.T).astype(bf)
    wkT = np.ascontiguousarray(k_w.T).astype(bf)
    wvT = np.ascontiguousarray(v_w.T).astype(bf)
    wpT = np.ascontiguousarray(proj_w.T).astype(bf)
    qbc = np.ascontiguousarray(q_b.reshape(C, 1), dtype=np.float32)
    gwc = np.ascontiguousarray(gn_g.reshape(C, 1), dtype=np.float32)
    gbc = np.ascontiguousarray(gn_b.reshape(C, 1), dtype=np.float32)

    gsw = np.zeros((CT, P, NGROUPS), np.float32)
    for ci in range(CT):
        for c in range(P):
            gsw[ci, c, (ci * P + c) // GSIZE] = 1.0

    in_maps = []
    for core in range(8):
        b, h = core // 2, core % 2
        xroll = np.roll(xf[b], -NQ * h, axis=1) if h else xf[b]
        in_maps.append(
            {
                "xb": np.ascontiguousarray(xroll).astype(bf),
                "xr": np.ascontiguousarray(
                    xf[b][:, h * NQ : (h + 1) * NQ] + pbe[:, None]
                ),
                "wq": wqT,
                "wk": wkT,
                "wv": wvT,
                "wp": wpT,
                "qb": qbc,
                "gw": gwc,
                "gb": gbc,
                "gs": gsw,
            }
        )
    return in_maps


def kernel(**inputs):
    if "nc" not in _cache:
        _cache["nc"] = build_program()
    nc = _cache["nc"]

    in_maps = _prep_inputs(**{k: np.asarray(v) for k, v in inputs.items()})
    res = run_bass_kernel_spmd(nc, in_maps, core_ids=list(range(8)))

    B = inputs["x"].shape[0]
    outf = np.empty((B, C, N), np.float32)
    for core in range(8):
        b, h = core // 2, core % 2
        outf[b][:, h * NQ : (h + 1) * NQ] = res.results[core]["out"]
    return outf.reshape(inputs["x"].shape)


# revision 16
# speedup vs baseline: 1.7349x; 1.7349x over previous
"""AttnBlock (GroupNorm + single-head attention over HW pixels + proj + residual)
on 8 trn2 NeuronCores.

Sharding: core i handles batch b = i//2, query-half h = i%2 (2048 of 4096 pixels).
Each core recomputes GroupNorm and full K/V for its image (no collectives).
The host rolls the pixel axis per core so queries are always columns [0, 2048):
attention is permutation-invariant over keys and GroupNorm over pixels.

Math notes:
  - k_b drops out of softmax (it only adds a per-query constant to scores).
  - v_b and proj_b fold into the residual on the host:
      x + proj_w @ (attn @ (v + v_b)) + proj_b = x + proj_w @ (attn@v) + (proj_b + proj_w@v_b)
    because softmax rows sum to 1.
  - Scores are O(1) for this distribution, so exp() without max-subtraction is safe.
  - Softmax layout: S^T is computed (keys on partitions) so exp goes PSUM->SBUF on
    ScalarE with no transposes; key-sums come from a ones-vector matmul on the PE.
  - The 1/sum softmax normalization is deferred past the proj matmul (attention
    output is kept unnormalized in fp8 -- relative precision is scale-invariant)
    and applied at the residual-add stage, so the PE never waits on it.
  - All big matmuls run fp8e4m3 with DoubleRow (2 contraction planes per matmul);
    accumulation stays fp32 in PSUM and all softmax statistics are fp32.
  - x streams in pre-cast to fp8 directly into the DoubleRow layout; GroupNorm
    stats are computed from the fp8 values (error ~6%/sqrt(65536) on stats) and
    normalization is applied in place, split across VectorE and ScalarE.
"""

from contextlib import ExitStack

import ml_dtypes
import numpy as np

import concourse.bacc as bacc
import concourse.tile as tile
from concourse import mybir
from concourse.bass_utils import run_bass_kernel_spmd

BF16 = mybir.dt.bfloat16
F32 = mybir.dt.float32
FP8 = mybir.dt.float8e4
AX = mybir.AxisListType
OP = mybir.AluOpType
AF = mybir.ActivationFunctionType
DR = mybir.MatmulPerfMode.DoubleRow

C = 512
N = 4096
NQ = 2048  # queries per core
P = 128
CT = C // P  # 4 channel part-tiles
CG = CT // 2  # 2 DoubleRow channel groups
JT = N // P  # 32 key tiles
JG = JT // 2  # 16 DoubleRow key groups
NCH = NQ // 512  # 4 query chunks of 512
GSIZE = 16  # channels per group
NGROUPS = 32
EPS = 1e-6
SCALE = float(C) ** -0.5
NA = 2560  # stats columns handled by DVE bn_stats (rest go to ScalarE)

_cache = {}


def build_program():
    nc = bacc.Bacc("TRN2", target_bir_lowering=False, debug=False, num_devices=8)

    # x pre-cast to fp8, channel-plane layout: [ki, p, n] = x[128p + ki, n]
    xb = nc.declare_dram_parameter("xb", [P, CT, N], FP8, isOutput=False)
    xr = nc.declare_dram_parameter("xr", [C, NQ], F32, isOutput=False)
    # all 4 weights in one wall: [ki, 4*w + plane, o] = w_T[128*(plane)+ki, o]
    ww = nc.declare_dram_parameter("ww", [P, 4 * CT, C], FP8, isOutput=False)
    qb = nc.declare_dram_parameter("qb", [C, 1], F32, isOutput=False)
    gw = nc.declare_dram_parameter("gw", [C, 1], F32, isOutput=False)
    gb = nc.declare_dram_parameter("gb", [C, 1], F32, isOutput=False)
    # group selector, doubled along the last axis (for fused mean/rstd extract)
    gs = nc.declare_dram_parameter("gs", [CT, P, 2 * NGROUPS], F32, isOutput=False)
    out = nc.declare_dram_parameter("out", [C, NQ], F32, isOutput=True)

    with tile.TileContext(nc) as tc, ExitStack() as ctx:
        # ---- persistent tiles -------------------------------------------------
        wpool = ctx.enter_context(tc.tile_pool(name="w", bufs=1))
        hpool = ctx.enter_context(tc.tile_pool(name="h", bufs=1))
        kpool = ctx.enter_context(tc.tile_pool(name="k", bufs=CG))
        qpool = ctx.enter_context(tc.tile_pool(name="q", bufs=CG))
        vpool = ctx.enter_context(tc.tile_pool(name="v", bufs=JG))
        cpool = ctx.enter_context(tc.tile_pool(name="c", bufs=2))
        spool = ctx.enter_context(tc.tile_pool(name="s", bufs=4 * CT))

        # x rides the sync HWDGE ring alone, in two halves so stats can start
        # before the full image lands
        h8 = hpool.tile([P, CT, N], FP8, tag="h8")
        # per-plane loads split across both HWDGE rings so per-ci stats start
        # as each plane lands; weights follow x on the scalar ring
        nc.sync.dma_start(out=h8[:, 0, 0:2048], in_=xb[:, 0, 0:2048])
        nc.scalar.dma_start(out=h8[:, 2, 0:2048], in_=xb[:, 2, 0:2048])
        nc.sync.dma_start(out=h8[:, 0, 2048:N], in_=xb[:, 0, 2048:N])
        nc.scalar.dma_start(out=h8[:, 2, 2048:N], in_=xb[:, 2, 2048:N])
        nc.sync.dma_start(out=h8[:, 1, :], in_=xb[:, 1, :])
        nc.scalar.dma_start(out=h8[:, 3, :], in_=xb[:, 3, :])

        wall = wpool.tile([P, 4 * CT, C], FP8, tag="w")
        nc.scalar.dma_start(out=wall[:], in_=ww[:])

        def wsl(widx, g):  # DoubleRow lhsT plane pair for weight widx, group g
            return wall[:, 4 * widx + 2 * g : 4 * widx + 2 * g + 2, :]

        qbt, gwt, gbt = [], [], []
        for ci in range(CT):
            sl = slice(ci * P, (ci + 1) * P)
            t = spool.tile([P, 1], F32, tag="qbt")
            nc.gpsimd.dma_start(out=t[:], in_=qb[sl, :])
            qbt.append(t)
            t = spool.tile([P, 1], F32, tag="gwt")
            nc.gpsimd.dma_start(out=t[:], in_=gw[sl, :])
            gwt.append(t)
            t = spool.tile([P, 1], F32, tag="gbt")
            nc.gpsimd.dma_start(out=t[:], in_=gb[sl, :])
            gbt.append(t)

        # padded to 16 cols so the DoubleRow lhsT plane step is 16B-aligned
        ones8 = cpool.tile([P, 2, 16], FP8, tag="ones")
        nc.vector.memset(ones8, 1.0)
        ones1 = cpool.tile([1, P], F32, tag="ones1")
        nc.vector.memset(ones1, 1.0)

        kt8 = [kpool.tile([P, 2, N], FP8, tag="kt", name=f"kt{g}") for g in range(CG)]
        qt8 = [qpool.tile([P, 2, NQ], FP8, tag="qt", name=f"qt{g}") for g in range(CG)]
        vt8 = [vpool.tile([P, 2, C], FP8, tag="vt", name=f"vt{g}") for g in range(JG)]

        # ---- phase 1: GroupNorm, in place over h8 -----------------------------
        with tc.tile_pool(name="gnt", bufs=2) as gntpool, \
             tc.tile_pool(name="gns", bufs=16) as gnspool, \
             tc.tile_pool(name="gnp", bufs=1, space="PSUM") as gnpsum:
            gst = [None] * CT
            xsum, xsq = [None] * CT, [None] * CT
            for ci in (0, 2, 1, 3):
                hsl = h8[:, ci, :]
                g = gnspool.tile([P, 2 * NGROUPS], F32, tag="gst")
                nc.gpsimd.dma_start(out=g[:], in_=gs[ci, :, :])
                gst[ci] = g

                # hybrid stats: bn_stats (one pass, sum+var) on DVE for the
                # first NA cols; ACT Copy/Square+accum for the rest
                nbn = NA // 512
                bst = gnspool.tile([P, nbn, 6], F32, tag="bst")
                for bi in range(nbn):
                    nc.vector.bn_stats(out=bst[:, bi, :],
                                       in_=hsl[:, bi * 512 : (bi + 1) * 512])
                mv = gnspool.tile([P, 2], F32, tag="mv")
                nc.vector.bn_aggr(out=mv[:], in_=bst[:])

                sq = gntpool.tile([P, N - NA], BF16, tag="sq")
                sumb = gnspool.tile([P, 1], F32, tag="sumb")
                nc.scalar.activation(out=sq[:], in_=hsl[:, NA:N], func=AF.Copy,
                                     accum_out=sumb[:])
                sq2 = gntpool.tile([P, N - NA], BF16, tag="sq2")
                sqb = gnspool.tile([P, 1], F32, tag="sqb")
                nc.scalar.activation(out=sq2[:], in_=hsl[:, NA:N], func=AF.Square,
                                     accum_out=sqb[:])

                # sums = mean*NA + sumb ; sumsq = (var+mean^2)*NA + sqb
                xs = gnspool.tile([P, 1], F32, tag="xsum")
                nc.vector.tensor_scalar(out=xs[:], in0=mv[:, 0:1],
                                        scalar1=float(NA), scalar2=sumb[:],
                                        op0=OP.mult, op1=OP.add)
                xsum[ci] = xs
                s2 = gnspool.tile([P, 1], F32, tag="xsq")
                m2 = gnspool.tile([P, 1], F32, tag="m2")
                nc.vector.tensor_mul(out=m2[:], in0=mv[:, 0:1], in1=mv[:, 0:1])
                nc.vector.tensor_add(out=m2[:], in0=m2[:], in1=mv[:, 1:2])
                nc.vector.tensor_scalar(out=s2[:], in0=m2[:],
                                        scalar1=float(NA), scalar2=sqb[:],
                                        op0=OP.mult, op1=OP.add)
                xsq[ci] = s2

            psums = gnpsum.tile([1, NGROUPS], F32, tag="psums")
            psq = gnpsum.tile([1, NGROUPS], F32, tag="psq")
            for ci in range(CT):
                nc.tensor.matmul(psums[:], lhsT=xsum[ci][:],
                                 rhs=gst[ci][:, 0:NGROUPS],
                                 start=(ci == 0), stop=(ci == CT - 1))
            for ci in range(CT):
                nc.tensor.matmul(psq[:], lhsT=xsq[ci][:],
                                 rhs=gst[ci][:, 0:NGROUPS],
                                 start=(ci == 0), stop=(ci == CT - 1))

            inv_n = 1.0 / (GSIZE * N)
            srow = gnspool.tile([1, 2 * NGROUPS], F32, tag="srow")
            mean = srow[:, 0:NGROUPS]
            rstd = srow[:, NGROUPS : 2 * NGROUPS]
            nc.vector.tensor_scalar_mul(out=mean, in0=psums[:], scalar1=inv_n)
            nc.vector.tensor_scalar_mul(out=rstd, in0=psq[:], scalar1=inv_n)
            msq = gnspool.tile([1, NGROUPS], F32, tag="msq")
            nc.vector.tensor_mul(out=msq[:], in0=mean, in1=mean)
            nc.vector.tensor_sub(out=rstd, in0=rstd, in1=msq[:])
            epst = gnspool.tile([1, 1], F32, tag="epst")
            nc.vector.memset(epst, EPS)
            nc.scalar.activation(out=rstd, in_=rstd, func=AF.Sqrt, bias=epst[:])
            nc.vector.reciprocal(out=rstd, in_=rstd)

            # broadcast [1, 64] stats row to all partitions via a K=1 matmul
            psb = gnpsum.tile([P, 2 * NGROUPS], F32, tag="psb")
            nc.tensor.matmul(psb[:], lhsT=ones1[:], rhs=srow[:],
                             start=True, stop=True)

            for ci in range(CT):
                hsl = h8[:, ci, :]
                jnk = gnspool.tile([P, 2 * NGROUPS], F32, tag="jnk")
                nc.vector.tensor_mul(out=jnk[:], in0=psb[:], in1=gst[ci][:])
                ms = gnspool.tile([P, 2], F32, tag="ms")
                nc.vector.reduce_sum(
                    out=ms[:], in_=jnk.rearrange("p (a b) -> p a b", a=2),
                    axis=AX.X)
                # s = rstd*gamma ; t = beta - mean*s ; h = x*s + t
                sc = gnspool.tile([P, 1], F32, tag="sc")
                nc.vector.tensor_mul(out=sc[:], in0=ms[:, 1:2], in1=gwt[ci][:])
                u = gnspool.tile([P, 1], F32, tag="u")
                nc.vector.tensor_mul(out=u[:], in0=ms[:, 0:1], in1=sc[:])
                tb = gnspool.tile([P, 1], F32, tag="tb")
                nc.vector.tensor_sub(out=tb[:], in0=gbt[ci][:], in1=u[:])
                # normalize in place, split across VectorE / ScalarE
                nc.vector.tensor_scalar(
                    out=hsl[:, 0 : N // 2], in0=hsl[:, 0 : N // 2],
                    scalar1=sc[:], scalar2=tb[:], op0=OP.mult, op1=OP.add)
                nc.scalar.activation(
                    out=hsl[:, N // 2 : N], in_=hsl[:, N // 2 : N],
                    func=AF.Identity, bias=tb[:], scale=sc[:])

        def hdr(g):  # DoubleRow rhs/lhsT plane pair of h for channel group g
            return h8[:, 2 * g : 2 * g + 2, :]

        # ---- phase 2: Q/K/V projections (fp8 DoubleRow, paired-bank copies) ---
        with tc.tile_pool(name="pqkv", bufs=4, space="PSUM") as pqkv:
            for og in range(CG):  # kt: pair the two oi of group og in one psum
                for ni in range(N // 512):
                    nsl = slice(ni * 512, (ni + 1) * 512)
                    ps = pqkv.tile([P, 2, 512], F32, tag="ps")
                    for s in range(2):
                        osl = slice((2 * og + s) * P, (2 * og + s + 1) * P)
                        for g in range(CG):
                            nc.tensor.matmul(ps[:, s, :], lhsT=wsl(1, g)[:, :, osl],
                                             rhs=hdr(g)[:, :, nsl], perf_mode=DR,
                                             start=(g == 0), stop=(g == CG - 1))
                    if ni % 2 == 0:
                        nc.vector.tensor_copy(out=kt8[og][:, :, nsl], in_=ps[:])
                    else:
                        nc.scalar.copy(out=kt8[og][:, :, nsl], in_=ps[:])
            for og in range(CG):
                for ni in range(NCH):
                    nsl = slice(ni * 512, (ni + 1) * 512)
                    ps = pqkv.tile([P, 2, 512], F32, tag="ps")
                    for s in range(2):
                        osl = slice((2 * og + s) * P, (2 * og + s + 1) * P)
                        for g in range(CG):
                            nc.tensor.matmul(ps[:, s, :], lhsT=wsl(0, g)[:, :, osl],
                                             rhs=hdr(g)[:, :, nsl], perf_mode=DR,
                                             start=(g == 0), stop=(g == CG - 1))
                        nc.vector.tensor_scalar_add(
                            out=qt8[og][:, s, nsl], in0=ps[:, s, :],
                            scalar1=qbt[2 * og + s][:])
            for jg in range(JG):
                ps = pqkv.tile([P, 2, 512], F32, tag="ps")
                for s in range(2):
                    jsl = slice((2 * jg + s) * P, (2 * jg + s + 1) * P)
                    for g in range(CG):
                        nc.tensor.matmul(ps[:, s, :], lhsT=hdr(g)[:, :, jsl],
                                         rhs=wsl(2, g)[:], perf_mode=DR,
                                         start=(g == 0), stop=(g == CG - 1))
                if jg % 3 == 2:
                    nc.scalar.copy(out=vt8[jg][:], in_=ps[:])
                else:
                    nc.vector.tensor_copy(out=vt8[jg][:], in_=ps[:])

        # ---- phase 3: attention + proj + residual ----------------------------
        with tc.tile_pool(name="xrp", bufs=CT) as xrpool, \
             tc.tile_pool(name="et", bufs=JG) as epool, \
             tc.tile_pool(name="at", bufs=2 * CG) as apool, \
             tc.tile_pool(name="ot", bufs=4) as opool, \
             tc.tile_pool(name="rc", bufs=4) as rcpool, \
             tc.tile_pool(name="pss", bufs=3, space="PSUM") as pss_pool, \
             tc.tile_pool(name="pcs", bufs=1, space="PSUM") as pcs_pool, \
             tc.tile_pool(name="pso", bufs=1, space="PSUM") as pso_pool, \
             tc.tile_pool(name="psp", bufs=1, space="PSUM") as psp_pool:

            xrt = []
            for ci in range(CT):
                t = xrpool.tile([P, NQ], F32, tag="xrt")
                nc.gpsimd.dma_start(out=t[:], in_=xr[ci * P : (ci + 1) * P, :])
                xrt.append(t)

            for ch in range(NCH):
                isl = slice(ch * 512, (ch + 1) * 512)

                et8 = [epool.tile([P, 2, 512], FP8, tag="et", name=f"et{ch}_{jg}")
                       for jg in range(JG)]
                pcs = pcs_pool.tile([1, 512], F32, tag="pcs")

                def colsum(jg):
                    nc.tensor.matmul(pcs[:], lhsT=ones8[:, :, 0:1], rhs=et8[jg][:],
                                     perf_mode=DR,
                                     start=(jg == 0), stop=(jg == JG - 1))

                for ji in range(JT):
                    jsl = slice(ji * P, (ji + 1) * P)
                    ps = pss_pool.tile([P, 512], F32, tag="pss")
                    for g in range(CG):
                        nc.tensor.matmul(ps[:], lhsT=kt8[g][:, :, jsl],
                                         rhs=qt8[g][:, :, isl], perf_mode=DR,
                                         start=(g == 0), stop=(g == CG - 1))
                    nc.scalar.activation(out=et8[ji // 2][:, ji % 2, :], in_=ps[:],
                                         func=AF.Exp, scale=SCALE)
                    # trail the S^T stream with colsum matmuls so the reciprocal
                    # chain completes during PV
                    if ji >= 5 and ji % 2 == 1:
                        colsum((ji - 5) // 2)
                for jg in range(JG - 3, JG):
                    colsum(jg)

                rc = rcpool.tile([1, 512], F32, tag="rc")
                nc.vector.reciprocal(out=rc[:], in_=pcs[:])
                rcb = rcpool.tile([P, 512], F32, tag="rcb")
                nc.gpsimd.partition_broadcast(rcb[:], rc[:], channels=P)

                # PV with both oi of a group paired into one 2-bank psum;
                # at8 is copied UNNORMALIZED (scale folded in at the ot stage)
                at8 = [apool.tile([P, 2, 512], FP8, tag="at", name=f"at{ch}_{g}")
                       for g in range(CG)]
                last = ch == NCH - 1
                for og in range(CG):
                    ps = pso_pool.tile([P, 2, 512], F32, tag="pso")
                    for s in range(2):
                        osl = slice((2 * og + s) * P, (2 * og + s + 1) * P)
                        for jg in range(JG):
                            nc.tensor.matmul(ps[:, s, :],
                                             lhsT=vt8[jg][:, :, osl],
                                             rhs=et8[jg][:], perf_mode=DR,
                                             start=(jg == 0), stop=(jg == JG - 1))
                    if last:
                        # normalize here so the final OT stage is one op shorter
                        for s in range(2):
                            nc.vector.tensor_mul(out=at8[og][:, s, :],
                                                 in0=ps[:, s, :], in1=rcb[:])
                    else:
                        nc.vector.tensor_copy(out=at8[og][:], in_=ps[:])

                for og in range(CG):
                    ps = psp_pool.tile([P, 2, 512], F32, tag="psp")
                    for s in range(2):
                        osl = slice((2 * og + s) * P, (2 * og + s + 1) * P)
                        for g in range(CG):
                            nc.tensor.matmul(ps[:, s, :], lhsT=wsl(3, g)[:, :, osl],
                                             rhs=at8[g][:], perf_mode=DR,
                                             start=(g == 0), stop=(g == CG - 1))
                    for s in range(2):
                        oi = 2 * og + s
                        o = opool.tile([P, 512], F32, tag="ot")
                        if last:
                            nc.vector.tensor_add(out=o[:], in0=ps[:, s, :],
                                                 in1=xrt[oi][:, isl])
                        else:
                            nc.vector.tensor_mul(out=o[:], in0=ps[:, s, :],
                                                 in1=rcb[:])
                            nc.vector.tensor_add(out=o[:], in0=o[:],
                                                 in1=xrt[oi][:, isl])
                        eng = nc.sync if oi % 2 == 0 else nc.scalar
                        eng.dma_start(out=out[oi * P : (oi + 1) * P, isl],
                                      in_=o[:])

    nc.compile()
    return nc


def _prep_inputs(x, gn_g, gn_b, q_w, q_b, k_w, k_b, v_w, v_b, proj_w, proj_b):
    B = x.shape[0]
    xf = np.ascontiguousarray(x.reshape(B, C, N), dtype=np.float32)
    pbe = (proj_b + proj_w.astype(np.float64) @ v_b.astype(np.float64)).astype(
        np.float32
    )

    # weight wall [ki, 4*widx + plane, o] = w.T[128*plane + ki, o], fp8
    wallw = np.empty((P, 4 * CT, C), np.float32)
    for widx, w in enumerate((q_w, k_w, v_w, proj_w)):
        wT = np.ascontiguousarray(w.T)  # [cin, cout]
        wallw[:, 4 * widx : 4 * widx + 4, :] = wT.reshape(CT, P, C).transpose(1, 0, 2)
    wall8 = wallw.astype(ml_dtypes.float8_e4m3)

    qbc = np.ascontiguousarray(q_b.reshape(C, 1), dtype=np.float32)
    gwc = np.ascontiguousarray(gn_g.reshape(C, 1), dtype=np.float32)
    gbc = np.ascontiguousarray(gn_b.reshape(C, 1), dtype=np.float32)

    gsw = np.zeros((CT, P, 2 * NGROUPS), np.float32)
    for ci in range(CT):
        for c in range(P):
            g = (ci * P + c) // GSIZE
            gsw[ci, c, g] = 1.0
            gsw[ci, c, NGROUPS + g] = 1.0

    in_maps = []
    for core in range(8):
        b, h = core // 2, core % 2
        xroll = np.roll(xf[b], -NQ * h, axis=1) if h else xf[b]
        # fp8 x in channel-plane layout [ki, plane, n]
        x8 = np.ascontiguousarray(
            xroll.reshape(CT, P, N).transpose(1, 0, 2)
        ).astype(ml_dtypes.float8_e4m3)
        in_maps.append(
            {
                "xb": x8,
                "xr": np.ascontiguousarray(
                    xf[b][:, h * NQ : (h + 1) * NQ] + pbe[:, None]
                ),
                "ww": wall8,
                "qb": qbc,
                "gw": gwc,
                "gb": gbc,
                "gs": gsw,
            }
        )
    return in_maps


def kernel(**inputs):
    if "nc" not in _cache:
        _cache["nc"] = build_program()
    nc = _cache["nc"]

    in_maps = _prep_inputs(**{k: np.asarray(v) for k, v in inputs.items()})
    res = run_bass_kernel_spmd(nc, in_maps, core_ids=list(range(8)))

    B = inputs["x"].shape[0]
    outf = np.empty((B, C, N), np.float32)
    for core in range(8):
        b, h = core // 2, core % 2
        outf[b][:, h * NQ : (h + 1) * NQ] = res.results[core]["out"]
    return outf.reshape(inputs["x"].shape)
